# revision 1
# baseline (speedup 1.0000x reference)
"""CapsuleLayer dynamic-routing kernel for 8 Trainium2 NeuronCores.

Problem: inputs [64, 4096, 8] f32, W [32, 4096, 16, 8] f32.
  hat[b,c,n,j] = sum_i W[c,n,j,i] * x[b,n,i]
  3 routing iterations: c = softmax_C(b); out = squash(sum_n c*hat);
  b += <out, hat>_j.

Strategy: shard the n (input-capsule) axis across the 8 cores
(N_loc = 512/core); everything stays SBUF-resident and hat is never
materialized.  Since the logit update is linear in out,
b_t = <sum_{t'<t} out_t', hat>, so logits are recomputed from the
running outsum each iteration.  Per routing iteration:

  - M1': A^T[(n,i), (c,b)] = <outsum, W> via matmuls with the W2 chunk
    ([(8c,16j) x 128(n,i)]) STATIONARY and the block-diagonal outsum
    tile moving (the cost model charges only moving columns; LDWEIGHTS
    is free), PSUM-drained to fp8 SBUF on a rotating ACT/DVE schedule.
  - M2: beta^T[nn, (c,b)] = sum_i A^T * x via fp8 DoubleRow matmuls
    whose stationary operand is a block-diagonal x tile: K covers one
    128-wide (16n x 8i) chunk, the lhsT maps it to the 16 n-rows, and
    chunk pairs ride the two DoubleRow k-tiles.  The per-(b,nt,pair)
    lhsT windows overlap in SBUF ([x0 | zeros | x1], stride-16 k-tile
    dim) to cut the zero padding.  This replaces the whole
    drain-multiply-tree pipeline of the classic formulation; the only
    per-element engine cost of the logits is the single fp8 drain.
  - exp straight from the beta^T PSUM into the transposed e-table
    (softmax over capsules needs no max-subtraction; logits are O(0.2));
    Z = sum_c e via an in-place bf16 add-tree; 1/Z is folded into x.
  - s partial = sum_n coupling*hat via matmuls with Rg = e*x~ (bf16,
    DVE/Pool) STATIONARY and 16-column W3 slices moving -> out [64b,16j]
    lands directly in the [b,(c,j)] layout squash wants; one AllReduce
    per iteration on that layout; squash (Newton-refined sqrt) runs
    identically on every core.

fp8 is confined to the LOGITS path (W2, A^T, x-diag): couplings only
need ~1% accuracy.  The final-output path (W3, Rg, s) stays bf16/f32.
"""

import numpy as np

B, N, I = 64, 4096, 8
C, D = 32, 16
ROUTINGS = 3
EPS = 1e-7
NCORES = 8
NL = N // NCORES          # 512 n per core
NT = NL // 128            # 4 partition tiles of n
NCH = NL * I // 128       # 32 chunks of 128 along flat (n,i)


# ---------------------------------------------------------------------------
# Host-side layout prep (pure numpy, per core)
# ---------------------------------------------------------------------------

def _xbd_pack(xk):
    """x block-diag windows [128=(dn,i), b, nt, q, 48] for DoubleRow M2.

    Window layout per (b, nt, q): [x_even(16) | zeros(16) | x_odd(16)];
    k-tile 0 reads cols 0:32, k-tile 1 reads cols 16:48 (stride-16
    overlap).  x_kt[dn*8+i, n'] = x[b, 128nt+32q+16kt+dn, i] * (dn==n').
    """
    import ml_dtypes
    arr = xk.reshape(B, NT, 4, 2, 16, I)     # [b, nt, q, kt, dn, i]
    out = np.zeros((16, I, NT, B, 4, 48), np.float32)
    for dn in range(16):
        out[dn, :, :, :, :, dn] = arr[:, :, :, 0, dn, :].transpose(3, 1, 0, 2)
        out[dn, :, :, :, :, 32 + dn] = arr[:, :, :, 1, dn, :].transpose(3, 1, 0, 2)
    return out.reshape(128, B * NT * 4 * 48).astype(ml_dtypes.float8_e4m3)


def host_prep(x, W, k):
    """Per-core input layouts for core k (n slice [k*NL, (k+1)*NL))."""
    n0 = k * NL
    Wk = np.ascontiguousarray(W[:, n0:n0 + NL])          # [C, NL, D, I]
    xk = np.ascontiguousarray(x[:, n0:n0 + NL])          # [B, NL, I]

    # W2 [128=(cp*16+j), (cg, n*8+i)]  = W[cg*8+cp, n, j, i]   (fp8e4)
    w2 = Wk.reshape(4, 8, NL, D, I).transpose(1, 3, 0, 2, 4).reshape(128, 4 * NL * I)
    # W3 [128=nn, (cb, nt, i, c8, j)] = W[cb*8+c8, nt*128+nn, j, i]  (bf16)
    w3 = Wk.reshape(4, 8, NT, 128, D, I).transpose(3, 0, 2, 5, 1, 4)            .reshape(128, NT * I * C * D)
    # xt3 [128=nn, (i, nt, b)] = x[b, nt*128+nn, i]             (bf16)
    xt3 = xk.reshape(B, NT, 128, I).transpose(2, 3, 1, 0).reshape(128, I * NT * B)

    import ml_dtypes
    bf = ml_dtypes.bfloat16
    f8 = ml_dtypes.float8_e4m3
    return {
        "w2": w2.astype(f8),
        "w3": w3.astype(bf),
        "xt3": xt3.astype(bf),
        "xbd": _xbd_pack(xk),
        "eyef": np.eye(128, dtype=np.float32),
        "bdmask": _bd_mask().astype(bf),
    }


_CONSTS = {}


def _prep_consts():
    if not _CONSTS:
        import ml_dtypes
        _CONSTS["eyef"] = np.eye(128, dtype=np.float32)
        _CONSTS["bdmask"] = _bd_mask().astype(ml_dtypes.bfloat16)
    return _CONSTS


def host_prep_all(x, W):
    """Vectorized host_prep for all cores at once."""
    import ml_dtypes
    bf = ml_dtypes.bfloat16
    f8 = ml_dtypes.float8_e4m3
    Wb = np.ascontiguousarray(W, dtype=np.float32).astype(bf)   # [C, N, D, I]
    xb = np.ascontiguousarray(x, dtype=np.float32).astype(bf)   # [B, N, I]
    K = NCORES
    w2 = Wb.reshape(4, 8, K, NL, D, I).transpose(2, 1, 4, 0, 3, 5)            .reshape(K, 128, 4 * NL * I)
    w3 = Wb.reshape(4, 8, K, NT, 128, D, I).transpose(2, 4, 0, 3, 6, 1, 5)            .reshape(K, 128, NT * I * C * D)
    xt3 = xb.reshape(B, K, NT, 128, I).transpose(1, 3, 4, 2, 0)             .reshape(K, 128, I * NT * B)
    xf = np.ascontiguousarray(x, dtype=np.float32)
    cst = _prep_consts()
    return [
        {"w2": np.ascontiguousarray(w2[k]).astype(f8),
         "w3": np.ascontiguousarray(w3[k]),
         "xt3": np.ascontiguousarray(xt3[k]),
         "xbd": _xbd_pack(np.ascontiguousarray(xf[:, k * NL:(k + 1) * NL])),
         "eyef": cst["eyef"], "bdmask": cst["bdmask"]}
        for k in range(K)
    ]


def _bd_mask():
    # mask[r, pp, col] = 1 where pp == r//32 and ((r%32)//16) == col//64 —
    # selects the p-block and b-half a 16-row (one capsule's j-block) feeds,
    # so one DVE op builds all four p-tiles of a capsule group.
    r = np.arange(128)[:, None, None]
    pp = np.arange(4)[None, :, None]
    col = np.arange(128)[None, None, :]
    m = (pp == r // 32) & (((r % 32) // 16) == (col // 64))
    return m.astype(np.float32).reshape(128, 512)


# ---------------------------------------------------------------------------
# Bass device program
# ---------------------------------------------------------------------------

_CACHE = {}

# rotating engine schedule for the 128 A^T PSUM->fp8 drains per iteration
# (GPSIMD cannot access PSUM, so only ACT / DVE qualify)
DRAIN_SCHED = "AVVAVVV"
PSA_BUFS = 4
PSB_BUFS = 2


def _build_nc(sim=False, ablate=()):
    import concourse.bass as bass
    import concourse.bacc as bacc
    import concourse.mybir as mybir
    import concourse.tile as tile

    dt = mybir.dt
    f32, bf16, f8e4 = dt.float32, dt.bfloat16, dt.float8e4
    ALU = mybir.AluOpType
    AF = mybir.ActivationFunctionType
    AX = mybir.AxisListType
    DR = mybir.MatmulPerfMode.DoubleRow

    nc = bacc.Bacc("TRN2", target_bir_lowering=False, debug=False,
                   num_devices=NCORES)

    w2_d = nc.dram_tensor("w2", [128, 4 * NL * I], f8e4, kind="ExternalInput").ap()
    w3_d = nc.dram_tensor("w3", [128, NT * I * C * D], bf16, kind="ExternalInput").ap()
    xt3_d = nc.dram_tensor("xt3", [128, I * NT * B], bf16, kind="ExternalInput").ap()
    xbd_d = nc.dram_tensor("xbd", [128, B * NT * 4 * 48], f8e4,
                           kind="ExternalInput").ap()
    eyef_d = nc.dram_tensor("eyef", [128, 128], f32, kind="ExternalInput").ap()
    bdm_d = nc.dram_tensor("bdmask", [128, 512], bf16, kind="ExternalInput").ap()
    out_d = nc.dram_tensor("out", [B, C * D], f32, kind="ExternalOutput").ap()

    with tile.TileContext(nc) as tc:
        with (
            tc.tile_pool(name="const", bufs=1) as cp,
            tc.tile_pool(name="work", bufs=2) as wp,
            tc.tile_pool(name="dram", bufs=2, space="DRAM") as dp,
        ):
            sW2 = cp.tile([128, 4, NCH, 128], f8e4)
            sW3 = cp.tile([128, 4, NT, I, 8, D], bf16)
            sXT3 = cp.tile([128, I, NT, B], bf16)
            sXBD = cp.tile([128, NT, B, 4, 48], f8e4)
            sEyeF = cp.tile([128, 128], f32)
            sBdm = cp.tile([128, 4, 128], bf16)
            # DMA order: t=0 needs W3 quarters + xt3 first; W2 by the t=1
            # M1' (~35us), xbd by the t=1 M2 (~55us).  All on SP/ACT
            # hardware-DGE queues (Pool never blocks on descriptors).
            w3v = sW3[:].rearrange("p cb a b c d -> p cb (a b c d)")
            qsz = NT * I * 8 * D
            nc.scalar.dma_start(sEyeF[:], eyef_d[:])
            nc.sync.dma_start(w3v[:, 0, :], w3_d[:, 0:qsz])
            nc.scalar.dma_start(sXT3[:].rearrange("p a b c -> p (a b c)"), xt3_d[:])
            for cbq in range(1, 4):
                nc.sync.dma_start(w3v[:, cbq, :],
                                  w3_d[:, cbq * qsz:(cbq + 1) * qsz])
            nc.scalar.dma_start(sBdm[:].rearrange("p a b -> p (a b)"), bdm_d[:])
            nc.sync.dma_start(sW2[:].rearrange("p a b c -> p (a b c)"), w2_d[:])
            # xbd is big (6.3MB) and only needed by the t=1 M2: keep it
            # behind W3/W2 on the same queue and split per nt-tile so the
            # first M2 only waits for a quarter of it
            xbdv = sXBD[:].rearrange("p a b c d -> p a (b c d)")
            xqsz = B * 4 * 48
            for ntq in range(NT):
                nc.sync.dma_start(xbdv[:, ntq, :],
                                  xbd_d[:, ntq * xqsz:(ntq + 1) * xqsz])

            sET = cp.tile([128, NT, C, B], bf16)
            sZt = cp.tile([128, NT, 8, B], bf16)
            sZ = cp.tile([128, NT, B], bf16)
            sZr = sZ
            sAT = cp.tile([128, NCH, 16, B], f8e4)      # per c-half, reused
            sST = cp.tile([128, 4, B], f32)
            sSTr = sST if sim else cp.tile([128, 4, B], f32)
            sSpre = cp.tile([B, C * D], f32)
            sS = cp.tile([B, C * D], f32)
            sOut = cp.tile([B, C * D], f32)
            sOsum = cp.tile([B, C * D], f32)
            sOsumT = cp.tile([128, 4, B], bf16)
            sBDall = cp.tile([128, 16, 128], bf16)
            sRg = cp.tile([128, I, NT, 8, B], bf16)
            # squash temps
            s2 = cp.tile([B, C], f32)
            s2e = cp.tile([B, C], f32)
            q = cp.tile([B, C], f32)
            rq = cp.tile([B, C], f32)
            q2 = cp.tile([B, C], f32)
            qs = cp.tile([B, C], f32)
            opp = cp.tile([B, C], f32)
            den = cp.tile([B, C], f32)
            rden = cp.tile([B, C], f32)
            fac = cp.tile([B, C], f32)

            nc.vector.memset(sBDall[:], 0.0)

            # prebuild the overlapping-window DoubleRow lhsT APs for M2
            VP = None
            xbd_ap = {}
            for b in range(B):
                for nt in range(NT):
                    for qq in range(4):
                        a = sXBD[:, nt, b, qq, :].copy()
                        if VP is None:
                            VP = type(a.ap)
                        part = list(a.ap)[0]
                        a.ap = VP([list(part), [16, 2], [1, 32]])
                        xbd_ap[(b, nt, qq)] = a

            def squash(src, dst):
                # fully per-capsule chain, in halves so consumers of dst's
                # leading slices start before the second half finishes
                for h in range(2):
                    sl = slice(h * C * D // 2, (h + 1) * C * D // 2)
                    cs = slice(h * C // 2, (h + 1) * C // 2)
                    nc.vector.tensor_mul(sSpre[:, sl], src[:, sl], src[:, sl])
                    nc.vector.tensor_reduce(
                        s2[:, cs],
                        sSpre[:, sl].rearrange("b (c j) -> b c j", j=D),
                        axis=AX.X, op=ALU.add)
                    nc.vector.tensor_scalar_add(s2e[:, cs], s2[:, cs], EPS)
                    nc.scalar.sqrt(q[:, cs], s2e[:, cs])
                    nc.vector.reciprocal(rq[:, cs], q[:, cs])
                    nc.vector.tensor_mul(q2[:, cs], s2e[:, cs], rq[:, cs])
                    nc.vector.tensor_add(qs[:, cs], q[:, cs], q2[:, cs])
                    nc.vector.tensor_scalar_add(opp[:, cs], s2[:, cs], 1.0)
                    nc.vector.tensor_mul(den[:, cs], opp[:, cs], qs[:, cs])
                    nc.vector.reciprocal(rden[:, cs], den[:, cs])
                    nc.vector.tensor_mul(fac[:, cs], s2[:, cs], rden[:, cs])
                    nc.vector.tensor_scalar_mul(fac[:, cs], fac[:, cs], 2.0)
                    fb = fac[:, cs].rearrange("b (c o) -> b c o", o=1) \
                        .broadcast_to([B, C // 2, D])
                    nc.vector.tensor_mul(
                        dst[:, sl].rearrange("b (c j) -> b c j", j=D),
                        src[:, sl].rearrange("b (c j) -> b c j", j=D), fb)

            n_rout = 1 if "r1" in ablate else (2 if "r2" in ablate else ROUTINGS)
            for t in range(n_rout):
                if t > 0:
                    # (a) transpose outsum, build block-diagonal lhsT tiles
                    with tc.tile_pool(name="psO", bufs=4, space="PSUM") as psO:
                        for m in range(4):
                            pT = psO.tile([128, B], f32, tag="ot")
                            nc.tensor.transpose(
                                pT[:], sOsum[:, 128 * m:128 * (m + 1)],
                                sEyeF[0:B, 0:B])
                            nc.scalar.copy(sOsumT[:, m, :], pT[:])
                    for g in range(4):
                        ob = sOsumT[:, g, :] \
                            .rearrange("p (o oo b) -> p o oo b", o=1, oo=1) \
                            .broadcast_to([128, 4, 2, B])
                        nc.vector.tensor_mul(
                            sBDall[:, g * 4:g * 4 + 4, :]
                                .rearrange("p a (h b) -> p a h b", h=2),
                            ob,
                            sBdm[:].rearrange("p a (h b) -> p a h b", h=2))
                    # (b) M1' (A^T) -> fp8 drains -> M2 (beta^T) -> exp
                    with (
                        tc.tile_pool(name="psA", bufs=PSA_BUFS, space="PSUM") as psA,
                        tc.tile_pool(name="psB", bufs=PSB_BUFS, space="PSUM") as psB,
                    ):
                        dr = 0
                        for h in range(2):
                            # nt-granular interleave: drain the two chunk
                            # groups an nt-tile needs, then immediately run
                            # its M2 + exp + scatter while the next groups
                            # drain
                            for nt in range(NT + 1):
                                if nt < NT:
                                    for cig in (2 * nt, 2 * nt + 1):
                                        for g in (2 * h, 2 * h + 1):
                                            for p in range(4):
                                                pA = psA.tile([128, 4, 128], f32,
                                                              tag="pA")
                                                for cc in range(4):
                                                    ci = cig * 4 + cc
                                                    nc.tensor.matmul(
                                                        pA[:, cc, :],
                                                        sW2[:, g, ci, :],
                                                        sBDall[:, g * 4 + p, :],
                                                        start=True, stop=True)
                                                clo = (g - 2 * h) * 8 + 2 * p
                                                dst = sAT[:, cig * 4:cig * 4 + 4,
                                                          clo:clo + 2, :]
                                                src = pA[:].rearrange(
                                                    "p a (c b) -> p a c b", c=2)
                                                v = DRAIN_SCHED[dr % len(DRAIN_SCHED)]
                                                dr += 1
                                                if v == "A":
                                                    nc.scalar.copy(dst, src)
                                                else:
                                                    nc.vector.tensor_copy(dst, src)
                                if nt == 0:
                                    continue
                                ntm = nt - 1
                                for qq in range(4):
                                    pB = psB.tile([32, B, 16], f32, tag="pB")
                                    for b in range(B):
                                        ci0 = 8 * ntm + 2 * qq
                                        rhs = sAT[:, ci0:ci0 + 2, :, b:b + 1] \
                                            .rearrange("p k c b -> p k (c b)")
                                        nc.tensor.matmul(
                                            pB[:, b, :],
                                            xbd_ap[(b, ntm, qq)], rhs,
                                            start=True, stop=True,
                                            perf_mode=DR)
                                    stg = wp.tile([32, D, B], bf16, tag="stg",
                                                  bufs=2)
                                    nc.scalar.activation(
                                        stg[:].rearrange("p c b -> p b c"),
                                        pB[:], AF.Exp)
                                    # HWDGE DMAs hold the issuing SEQ
                                    # through their waits; keep them off the
                                    # busy ACT queue (SP is idle)
                                    nc.sync.dma_start(
                                        sET[32 * qq:32 * qq + 32, ntm,
                                            16 * h:16 * h + 16, :], stg[:])
                    # (c) Z = sum_c e (in-place bf16 add-tree); x~ = xt3 / Z
                    ZT = sET[:].rearrange("p nt c b -> p nt c b")
                    nc.vector.tensor_add(sZt[:], ZT[:, :, 0:8, :], ZT[:, :, 8:16, :])
                    nc.vector.tensor_add(sZt[:], sZt[:], ZT[:, :, 16:24, :])
                    nc.vector.tensor_add(sZt[:], sZt[:], ZT[:, :, 24:32, :])
                    nc.vector.tensor_add(sZt[:, :, 0:4, :],
                                         sZt[:, :, 0:4, :], sZt[:, :, 4:8, :])
                    nc.vector.tensor_add(sZt[:, :, 0:2, :],
                                         sZt[:, :, 0:2, :], sZt[:, :, 2:4, :])
                    nc.vector.tensor_add(sZ[:].rearrange("p a b -> p a () b"),
                                         sZt[:, :, 0:1, :], sZt[:, :, 1:2, :])
                    with nc.allow_low_precision(reason="Z~32, bf16 1/Z only "
                                                "perturbs couplings"):
                        nc.vector.reciprocal(sZr[:], sZ[:])
                    zb = sZr[:].rearrange("p (o nt) b -> p o nt b", o=1) \
                        .broadcast_to([128, I, NT, B])
                    sXt = sZt[:].rearrange("p nt c b -> p c nt b")
                    nc.vector.tensor_mul(sXt, sXT3[:], zb)
                # (d) s matmuls
                for cb in range(4):
                    if t > 0:
                        for i in range(I):
                            xb = sZt[:].rearrange(
                                "p nt c b -> p c nt b")[:, i, :, :] \
                                .rearrange("p nt (o b) -> p nt o b", o=1) \
                                .broadcast_to([128, NT, 8, B])
                            # Pool takes the last-consumed i-slices so its
                            # slow ops overlap DVE's and finish before the
                            # PE stream reaches them
                            eng = nc.gpsimd if i == 7 or (i == 6 and cb % 2 == 0) \
                                else nc.vector
                            eng.tensor_mul(
                                sRg[:, i, :, :, :],
                                sET[:, :, cb * 8:(cb + 1) * 8, :], xb)
                    with tc.tile_pool(name=f"psS{t}{cb}", bufs=1, space="PSUM") as psS:
                        if t == 0:
                            pacc = psS.tile([128, B], f32, tag="s8")
                            step = 0
                            for i in range(I):
                                for nt in range(NT):
                                    lhs = sW3[:, cb, nt, i, :, :] \
                                        .rearrange("p a b -> p (a b)")
                                    nc.tensor.matmul(
                                        pacc[:], lhs, sXT3[:, i, nt, :],
                                        start=(step == 0), stop=(step == 31))
                                    step += 1
                            nc.scalar.mul(sST[:, cb, :], pacc[:], 1.0 / C)
                        else:
                            # flipped: Rg stationary, 16-col W3 moving ->
                            # out [64b, 16j]; one PSUM tile per c8.
                            paccs = [psS.tile([B, D], f32, name=f"pacc{c8}",
                                              tag=f"s{c8}")
                                     for c8 in range(8)]
                            step = 0
                            for i in range(I):
                                for nt in range(NT):
                                    for c8 in range(8):
                                        nc.tensor.matmul(
                                            paccs[c8][:],
                                            sRg[:, i, nt, c8, :],
                                            sW3[:, cb, nt, i, c8, :],
                                            start=(step == 0),
                                            stop=(step == 31))
                                    step += 1
                            for c8 in range(8):
                                nc.scalar.copy(
                                    sSpre[:, (cb * 8 + c8) * D:
                                          (cb * 8 + c8 + 1) * D],
                                    paccs[c8][:])
                # all-reduce partial s across cores; t=0 uses the transposed
                # sST layout (+ PE transposes back), t>0 reduces [64, 512].
                if t == 0:
                    if not sim:
                        di = dp.tile([128, 4 * B], f32, tag="ar_in")
                        do = dp.tile([128, 4 * B], f32, tag="ar_out")
                        nc.sync.dma_start(di[:], sST[:].rearrange("p a b -> p (a b)"))
                        nc.gpsimd.collective_compute(
                            "AllReduce", mybir.AluOpType.add,
                            replica_groups=[list(range(NCORES))],
                            ins=[di[:].opt()], outs=[do[:].opt()])
                        nc.sync.dma_start(sSTr[:].rearrange("p a b -> p (a b)"), do[:])
                    with tc.tile_pool(name=f"psT{t}", bufs=2, space="PSUM") as psT:
                        for cb in range(4):
                            pT3 = psT.tile([B, 128], f32, tag="sT")
                            nc.tensor.transpose(pT3[:], sSTr[:, cb, :], sEyeF[:])
                            nc.scalar.copy(sS[:, cb * 128:(cb + 1) * 128], pT3[:])
                else:
                    if sim:
                        nc.vector.tensor_copy(sS[:], sSpre[:])
                    else:
                        di2 = dp.tile([B, C * D], f32, tag="ar_in2")
                        do2 = dp.tile([B, C * D], f32, tag="ar_out2")
                        nc.sync.dma_start(di2[:], sSpre[:])
                        nc.gpsimd.collective_compute(
                            "AllReduce", mybir.AluOpType.add,
                            replica_groups=[list(range(NCORES))],
                            ins=[di2[:].opt()], outs=[do2[:].opt()])
                        nc.sync.dma_start(sS[:], do2[:])
                squash(sS, sOut)
                if t == n_rout - 1:
                    nc.sync.dma_start(out_d[:], sOut[:])
                elif t == 0:
                    for h in range(2):
                        sl = slice(h * C * D // 2, (h + 1) * C * D // 2)
                        nc.vector.tensor_copy(sOsum[:, sl], sOut[:, sl])
                else:
                    for h in range(2):
                        sl = slice(h * C * D // 2, (h + 1) * C * D // 2)
                        nc.vector.tensor_add(sOsum[:, sl], sOsum[:, sl],
                                             sOut[:, sl])
    nc.compile()
    return nc


def get_nc(sim=False, ablate=()):
    key = ("nc_sim" if sim else "nc") + "_".join(ablate)
    if key not in _CACHE:
        _CACHE[key] = _build_nc(sim=sim, ablate=ablate)
    return _CACHE[key]


def kernel(inputs, W):
    inputs = np.asarray(inputs, dtype=np.float32)
    W = np.asarray(W, dtype=np.float32)
    nc = get_nc()
    in_maps = host_prep_all(inputs, W)
    from concourse import bass_utils
    res = bass_utils.run_bass_kernel_spmd(
        nc, in_maps, core_ids=list(range(NCORES)))
    return res.results[0]["out"].reshape(B, C, D).astype(np.float32)



# revision 20
# speedup vs baseline: 1.0468x; 1.0468x over previous
"""CapsuleLayer dynamic-routing kernel for 8 Trainium2 NeuronCores.

Problem: inputs [64, 4096, 8] f32, W [32, 4096, 16, 8] f32.
  hat[b,c,n,j] = sum_i W[c,n,j,i] * x[b,n,i]
  3 routing iterations: c = softmax_C(b); out = squash(sum_n c*hat);
  b += <out, hat>_j.

Strategy: shard the n (input-capsule) axis across the 8 cores
(N_loc = 512/core); everything stays SBUF-resident and hat is never
materialized.  Since the logit update is linear in out,
b_t = <sum_{t'<t} out_t', hat>, so logits are recomputed from the
running outsum each iteration.

Per routing iteration, a single software-pipelined loop over
(nt, c-half) units fuses BOTH halves of the iteration:

  - M1': A^T[(n,i), (c,b)] = <outsum, W> via 256-col fp8 matmuls with
    the W2 chunk stationary and the block-diagonal outsum tile moving,
    PSUM-drained to fp8 SBUF in 1024-element chunks on a mostly-ACT
    schedule (ACT is the cheaper drain engine at 0.83ns/elem).
  - M2: beta^T[nn, (c,b)] = sum_i A^T * x via fp8 DoubleRow matmuls
    whose stationary operand is a block-diagonal x tile; all 4 q-blocks
    of an nt land in ONE 128-partition PSUM tile so a single exp per
    (nt, half) drains it straight into the transposed e-table.
  - softmax: Z = sum_c e via a 5-op in-place bf16 add-tree per nt;
    1/Z folded into x (x~); Rg = e * x~ built as 4 big broadcast DVE
    muls per nt (bf16 2x mode).
  - s: Rg stationary [128,64], 16-col W3 slices moving, accumulated
    over (nt, i) into one [64, 32, 16] PSUM tile -> drained once.

  The softmax/Rg/s work of tile nt overlaps the M1'/drain/M2 work of
  tile nt+1 (the old design ran them as separate serial phases with
  ACT and PE idle during the s-phase).

fp8 is confined to the LOGITS path (W2, A^T, x-diag): couplings only
need ~1% accuracy.  The final-output path (W3, Rg, s) stays bf16/f32.
One AllReduce per iteration on the partial s.
"""

import numpy as np

B, N, I = 64, 4096, 8
C, D = 32, 16
ROUTINGS = 3
EPS = 1e-7
NCORES = 8
NL = N // NCORES          # 512 n per core
NT = NL // 128            # 4 partition tiles of n
NCH = NL * I // 128       # 32 chunks of 128 along flat (n,i)


# ---------------------------------------------------------------------------
# Host-side layout prep (pure numpy, per core)
# ---------------------------------------------------------------------------

def _xbd_pack(xk):
    """x diagonal windows [128=(dn32,il4), nt, b, q, ih, 32] for M2.

    lhsT for (b, nt, q, ih) is [128=(dn*4+il), 32 n']: value
    x[b, 128nt+32q+dn, 4ih+il] at col n'=dn, zero elsewhere.  The two
    i-halves accumulate in PSUM (walrus only allows DoubleRow at column
    position 0, so the 32-row output blocks use plain fp8 matmuls at
    positions 0/32/64/96 instead).

    Eight dense [128, 32] windows per (b, nt), w = 2q + ih (a stride-16
    overlap would leak neighbour diagonals: a 32-row diagonal occupies
    every row, so windows cannot share columns).
    """
    import ml_dtypes
    arr = xk.reshape(B, NT, 4, 32, 2, 4)     # [b, nt, q, dn, ih, il]
    out = np.zeros((32, 4, NT, B, 8, 32), np.float32)
    for w in range(8):
        q, ih = w // 2, w % 2
        for dn in range(32):
            out[dn, :, :, :, w, dn] = arr[:, :, q, dn, ih, :] \
                .transpose(2, 1, 0)
    return out.reshape(128, NT * B * 256).astype(ml_dtypes.float8_e4m3)


def host_prep(x, W, k):
    """Per-core input layouts for core k (n slice [k*NL, (k+1)*NL))."""
    n0 = k * NL
    Wk = np.ascontiguousarray(W[:, n0:n0 + NL])          # [C, NL, D, I]
    xk = np.ascontiguousarray(x[:, n0:n0 + NL])          # [B, NL, I]

    # W2 [128=(cp*16+j), (cg, nb, ih, dn, il)] = W[cg*8+cp, nb*32+dn, j, 4ih+il]
    w2 = Wk.reshape(4, 8, 16, 32, D, 2, 4).transpose(1, 4, 0, 2, 5, 3, 6) \
        .reshape(128, 4 * NL * I)
    # W3 [128=nn, (cb, nt, i, c8, j)] = W[cb*8+c8, nt*128+nn, j, i]  (bf16)
    w3 = Wk.reshape(4, 8, NT, 128, D, I).transpose(3, 0, 2, 5, 1, 4)            .reshape(128, NT * I * C * D)
    # xt3 [128=nn, (i, nt, b)] = x[b, nt*128+nn, i]             (bf16)
    xt3 = xk.reshape(B, NT, 128, I).transpose(2, 3, 1, 0).reshape(128, I * NT * B)

    import ml_dtypes
    bf = ml_dtypes.bfloat16
    f8 = ml_dtypes.float8_e4m3
    return {
        "w2": w2.astype(f8),
        "w3": w3.astype(bf),
        "xt3": xt3.astype(bf),
        "xbd": _xbd_pack(xk),
        "eyef": np.eye(128, dtype=np.float32),
        "bdmask": _bd_mask().astype(bf),
    }


_CONSTS = {}


def _prep_consts():
    if not _CONSTS:
        import ml_dtypes
        _CONSTS["eyef"] = np.eye(128, dtype=np.float32)
        _CONSTS["bdmask"] = _bd_mask().astype(ml_dtypes.bfloat16)
    return _CONSTS


def host_prep_all(x, W):
    """Vectorized host_prep for all cores at once."""
    import ml_dtypes
    bf = ml_dtypes.bfloat16
    f8 = ml_dtypes.float8_e4m3
    Wb = np.ascontiguousarray(W, dtype=np.float32).astype(bf)   # [C, N, D, I]
    xb = np.ascontiguousarray(x, dtype=np.float32).astype(bf)   # [B, N, I]
    K = NCORES
    w2 = Wb.reshape(4, 8, K, 16, 32, D, 2, 4) \
        .transpose(2, 1, 5, 0, 3, 6, 4, 7).reshape(K, 128, 4 * NL * I)
    w3 = Wb.reshape(4, 8, K, NT, 128, D, I).transpose(2, 4, 0, 3, 6, 1, 5)            .reshape(K, 128, NT * I * C * D)
    xt3 = xb.reshape(B, K, NT, 128, I).transpose(1, 3, 4, 2, 0)             .reshape(K, 128, I * NT * B)
    xf = np.ascontiguousarray(x, dtype=np.float32)
    cst = _prep_consts()
    return [
        {"w2": np.ascontiguousarray(w2[k]).astype(f8),
         "w3": np.ascontiguousarray(w3[k]),
         "xt3": np.ascontiguousarray(xt3[k]),
         "xbd": _xbd_pack(np.ascontiguousarray(xf[:, k * NL:(k + 1) * NL])),
         "eyef": cst["eyef"], "bdmask": cst["bdmask"]}
        for k in range(K)
    ]


def _bd_mask():
    # mask[r, pp, col] = 1 where pp == r//32 and ((r%32)//16) == col//64 —
    # selects the p-block and b-half a 16-row (one capsule's j-block) feeds,
    # so one DVE op builds all four p-tiles of a capsule group.
    r = np.arange(128)[:, None, None]
    pp = np.arange(4)[None, :, None]
    col = np.arange(128)[None, None, :]
    m = (pp == r // 32) & (((r % 32) // 16) == (col // 64))
    return m.astype(np.float32).reshape(128, 512)


# ---------------------------------------------------------------------------
# Bass device program
# ---------------------------------------------------------------------------

_CACHE = {}

# engine schedule for the 64 A^T PSUM->fp8 drains per iteration
# (GPSIMD cannot access PSUM, so only ACT / DVE qualify; ACT is cheaper
# per element and DVE carries the Rg/Z work, so mostly-ACT)
DRAIN_SCHED = "AAAAVAAAVAAAAVAA"


def _build_nc(sim=False, ablate=()):
    import concourse.bass as bass
    import concourse.bacc as bacc
    import concourse.mybir as mybir
    import concourse.tile as tile

    dt = mybir.dt
    f32, bf16, f8e4 = dt.float32, dt.bfloat16, dt.float8e4
    ALU = mybir.AluOpType
    AF = mybir.ActivationFunctionType
    AX = mybir.AxisListType
    DR = mybir.MatmulPerfMode.DoubleRow

    nc = bacc.Bacc("TRN2", target_bir_lowering=False, debug=False,
                   num_devices=NCORES)

    w2_d = nc.dram_tensor("w2", [128, 4 * NL * I], f8e4, kind="ExternalInput").ap()
    w3_d = nc.dram_tensor("w3", [128, NT * I * C * D], bf16, kind="ExternalInput").ap()
    xt3_d = nc.dram_tensor("xt3", [128, I * NT * B], bf16, kind="ExternalInput").ap()
    xbd_d = nc.dram_tensor("xbd", [128, NT * B * 256], f8e4,
                           kind="ExternalInput").ap()
    eyef_d = nc.dram_tensor("eyef", [128, 128], f32, kind="ExternalInput").ap()
    bdm_d = nc.dram_tensor("bdmask", [128, 512], bf16, kind="ExternalInput").ap()
    out_d = nc.dram_tensor("out", [B, C * D], f32, kind="ExternalOutput").ap()
    if "dbg" in ablate:
        dbgE_d = nc.dram_tensor("dbgE", [128, NT * C * B], mybir.dt.bfloat16,
                                kind="ExternalOutput").ap()
        dbgS_d = nc.dram_tensor("dbgS", [B, C * D], f32,
                                kind="ExternalOutput").ap()

    with tile.TileContext(nc) as tc:
        with (
            tc.tile_pool(name="const", bufs=1) as cp,
            tc.tile_pool(name="dram", bufs=2, space="DRAM") as dp,
        ):
            sW2 = cp.tile([128, 4, NCH, 128], f8e4)
            sW3 = cp.tile([128, 4, NT, I, 8, D], bf16)
            sXT3 = cp.tile([128, I, NT, B], bf16)
            sXBD = cp.tile([128, NT, B, 8, 32], f8e4)
            sEyeF = cp.tile([128, 128], f32)
            sBdm = cp.tile([128, 4, 128], bf16)
            # DMA order: t=0 needs W3 quarters + xt3 first; W2 by the t=1
            # M1' and the xbd quarters by the t=1 M2s.  Split across the SP
            # and ACT hardware-DGE queues roughly evenly by need-time.
            w3v = sW3[:].rearrange("p cb a b c d -> p cb (a b c d)")
            qsz = NT * I * 8 * D
            xbdv = sXBD[:].rearrange("p a b c d -> p a (b c d)")
            xqsz = B * 256

            def w3q(cbq, q):
                q.dma_start(w3v[:, cbq, :], w3_d[:, cbq * qsz:(cbq + 1) * qsz])

            def xbdq(ntq, q):
                q.dma_start(xbdv[:, ntq, :],
                            xbd_d[:, ntq * xqsz:(ntq + 1) * xqsz])

            # W3 quarters split across both queues so t=0 unblocks fast;
            # then xbd0/1 + W2 (needed by the first t=1 units), rest last.
            nc.scalar.dma_start(sXT3[:].rearrange("p a b c -> p (a b c)"), xt3_d[:])
            w3q(0, nc.sync)
            w3q(1, nc.sync)
            nc.scalar.dma_start(sEyeF[:], eyef_d[:])
            nc.scalar.dma_start(sBdm[:].rearrange("p a b -> p (a b)"), bdm_d[:])
            w3q(2, nc.sync)
            w3q(3, nc.sync)
            nc.scalar.dma_start(sW2[:].rearrange("p a b c -> p (a b c)"), w2_d[:])
            xbdq(0, nc.sync)
            xbdq(1, nc.sync)
            xbdq(2, nc.scalar)
            xbdq(3, nc.scalar)

            sET = cp.tile([128, NT, C, B], bf16)
            sZt = cp.tile([128, 16, B], bf16)
            sZ = cp.tile([128, B], bf16)
            sZr = sZ
            sXtN = cp.tile([128, I, B], bf16)
            sRgN = [cp.tile([128, I, 8, B], bf16, name=f"sRgN{_cb}")
                    for _cb in range(4)]
            sAT = cp.tile([128, 2, 8, 16, B], f8e4)   # [slot=nt%2, chunk, c, b]
            sST = cp.tile([128, 4, B], f32)
            sSTr = sST if sim else cp.tile([128, 4, B], f32)
            sSpre = cp.tile([B, C * D], f32)
            sS = cp.tile([B, C * D], f32)
            sOut = cp.tile([B, C * D], f32)
            sOsum = cp.tile([B, C * D], f32)
            sOsumT = cp.tile([128, 4, B], bf16)
            sBDall = cp.tile([128, 16, 128], bf16)
            # squash temps
            s2 = cp.tile([B, C], f32)
            s2e = cp.tile([B, C], f32)
            q = cp.tile([B, C], f32)
            rq = cp.tile([B, C], f32)
            q2 = cp.tile([B, C], f32)
            qs = cp.tile([B, C], f32)
            opp = cp.tile([B, C], f32)
            den = cp.tile([B, C], f32)
            rden = cp.tile([B, C], f32)
            fac = cp.tile([B, C], f32)

            nc.vector.memset(sBDall[:], 0.0)


            def squash(src, dst):
                # fully per-capsule chain, in halves so consumers of dst's
                # leading slices start before the second half finishes
                for h in range(2):
                    sl = slice(h * C * D // 2, (h + 1) * C * D // 2)
                    cs = slice(h * C // 2, (h + 1) * C // 2)
                    nc.vector.tensor_mul(sSpre[:, sl], src[:, sl], src[:, sl])
                    nc.vector.tensor_reduce(
                        s2[:, cs],
                        sSpre[:, sl].rearrange("b (c j) -> b c j", j=D),
                        axis=AX.X, op=ALU.add)
                    nc.vector.tensor_scalar_add(s2e[:, cs], s2[:, cs], EPS)
                    nc.scalar.sqrt(q[:, cs], s2e[:, cs])
                    nc.vector.reciprocal(rq[:, cs], q[:, cs])
                    nc.vector.tensor_mul(q2[:, cs], s2e[:, cs], rq[:, cs])
                    nc.vector.tensor_add(qs[:, cs], q[:, cs], q2[:, cs])
                    nc.vector.tensor_scalar_add(opp[:, cs], s2[:, cs], 1.0)
                    nc.vector.tensor_mul(den[:, cs], opp[:, cs], qs[:, cs])
                    nc.vector.reciprocal(rden[:, cs], den[:, cs])
                    nc.vector.tensor_mul(fac[:, cs], s2[:, cs], rden[:, cs])
                    nc.vector.tensor_scalar_mul(fac[:, cs], fac[:, cs], 2.0)
                    fb = fac[:, cs].rearrange("b (c o) -> b c o", o=1) \
                        .broadcast_to([B, C // 2, D])
                    nc.vector.tensor_mul(
                        dst[:, sl].rearrange("b (c j) -> b c j", j=D),
                        src[:, sl].rearrange("b (c j) -> b c j", j=D), fb)

            n_rout = 1 if "r1" in ablate else (2 if "r2" in ablate else ROUTINGS)
            for t in range(n_rout):
                if t > 0:
                    # (a) transpose outsum, build block-diagonal lhsT tiles
                    with tc.tile_pool(name="psO", bufs=4, space="PSUM") as psO:
                        for m in range(4):
                            pT = psO.tile([128, B], f32, tag="ot")
                            nc.tensor.transpose(
                                pT[:], sOsum[:, 128 * m:128 * (m + 1)],
                                sEyeF[0:B, 0:B])
                            nc.scalar.copy(sOsumT[:, m, :], pT[:])
                    for g in range(4):
                        ob = sOsumT[:, g, :] \
                            .rearrange("p (o oo b) -> p o oo b", o=1, oo=1) \
                            .broadcast_to([128, 4, 2, B])
                        nc.vector.tensor_mul(
                            sBDall[:, g * 4:g * 4 + 4, :]
                                .rearrange("p a (h b) -> p a h b", h=2),
                            ob,
                            sBdm[:].rearrange("p a (h b) -> p a h b", h=2))
                    # (b) fused pipeline over units u = nt*2 + h:
                    #     M1'(u) || drains(u) || M2(u-1)+exp(u-1) ||
                    #     softmax/Rg/s(nt of u-3)
                    with (
                        tc.tile_pool(name="psA", bufs=2, space="PSUM") as psA,
                        tc.tile_pool(name="psB", bufs=2, space="PSUM") as psB,
                    ):
                        dr = 0
                        for u in range(12):
                            # (1) M2 + exp of the previous unit: its drain
                            # deps are met, so it issues immediately and the
                            # exp leads the ACT queue ahead of unit u's drains
                            if 1 <= u <= 8:
                                ntm, hm = (u - 1) // 2, (u - 1) % 2
                                pB = psB.tile([128, B, 16], f32, tag="pB",
                                              bufs=1)
                                for qq in range(4):
                                    for b in range(B):
                                        for ih in range(2):
                                            ci = 2 * qq + ih
                                            rhs = sAT[:, ntm % 2, ci, :,
                                                      b:b + 1] \
                                                .rearrange("p c b -> p (c b)")
                                            nc.tensor.matmul(
                                                pB[32 * qq:32 * qq + 32,
                                                   b, :],
                                                sXBD[:, ntm, b,
                                                     2 * qq + ih, :],
                                                rhs,
                                                start=(ih == 0),
                                                stop=(ih == 1),
                                                tile_position=(0, 32 * qq))
                                nc.scalar.activation(
                                    sET[:, ntm, 16 * hm:16 * hm + 16, :]
                                        .rearrange("p c b -> p b c"),
                                    pB[:], AF.Exp)
                            # (2) s-matmuls, two units behind their Rg build
                            # so they never stall the PE queue head
                            if u >= 5 and (u - 5) % 2 == 0:
                                nts = (u - 5) // 2
                                pacc = psB.tile([B, C, D], f32, tag="pacc", bufs=1)
                                for cb in range(4):
                                    for c8 in range(8):
                                        c = cb * 8 + c8
                                        for i in range(I):
                                            nc.tensor.matmul(
                                                pacc[:, c, :],
                                                sRgN[cb][:, i, c8, :],
                                                sW3[:, cb, nts, i, c8, :],
                                                start=(i == 0),
                                                stop=(i == 7))
                                if nts == 0:
                                    nc.vector.tensor_copy(
                                        sSpre[:],
                                        pacc[:].rearrange("b c j -> b (c j)"))
                                else:
                                    nc.vector.tensor_add(
                                        sSpre[:], sSpre[:],
                                        pacc[:].rearrange("b c j -> b (c j)"))
                            # (3) M1' + drains of unit u
                            if u < 8:
                                nt, h = u // 2, u % 2
                                for cig in (2 * nt, 2 * nt + 1):
                                    for g in (2 * h, 2 * h + 1):
                                        for cp in range(2):
                                            pA = psA.tile(
                                                [128, 2, 2, 2, 128],
                                                f32, tag="pA")
                                            for cc in range(2):
                                                ci = cig * 4 + cp * 2 + cc
                                                for pp in range(2):
                                                    mv = sBDall[
                                                        :, g * 4 + 2 * pp:
                                                        g * 4 + 2 * pp + 2, :]
                                                    nc.tensor.matmul(
                                                        pA[:, cc, pp, :, :],
                                                        sW2[:, g, ci, :],
                                                        mv, start=True,
                                                        stop=True)
                                            gl = (g - 2 * h) * 8
                                            ci0 = cig * 4 + cp * 2 - 8 * nt
                                            dst = sAT[:, nt % 2,
                                                      ci0:ci0 + 2,
                                                      gl:gl + 8, :]
                                            src = pA[:].rearrange(
                                                "p a q r (c b) -> p a (q r c) b",
                                                c=2)
                                            v = DRAIN_SCHED[
                                                dr % len(DRAIN_SCHED)]
                                            dr += 1
                                            if v == "A":
                                                nc.scalar.copy(dst, src)
                                            else:
                                                nc.vector.tensor_copy(
                                                    dst, src)
                            # (4) softmax chain + Rg build for the nt whose
                            # second half just finished exp
                            if u >= 3 and (u - 3) % 2 == 0 and (u - 3) // 2 < NT:
                                ntp = (u - 3) // 2
                                # Z = sum_c e (in-place bf16 add-tree)
                                e = sET[:, ntp, :, :]
                                nc.vector.tensor_add(sZt[:], e[:, 0:16, :],
                                                     e[:, 16:32, :])
                                nc.vector.tensor_add(sZt[:, 0:8, :],
                                                     sZt[:, 0:8, :],
                                                     sZt[:, 8:16, :])
                                nc.vector.tensor_add(sZt[:, 0:4, :],
                                                     sZt[:, 0:4, :],
                                                     sZt[:, 4:8, :])
                                nc.vector.tensor_add(sZt[:, 0:2, :],
                                                     sZt[:, 0:2, :],
                                                     sZt[:, 2:4, :])
                                nc.vector.tensor_add(
                                    sZ[:].rearrange("p b -> p () b"),
                                    sZt[:, 0:1, :], sZt[:, 1:2, :])
                                with nc.allow_low_precision(
                                        reason="Z~32, bf16 1/Z only perturbs "
                                        "couplings"):
                                    nc.vector.reciprocal(sZr[:], sZ[:])
                                zb = sZr[:].rearrange("p (o b) -> p o b", o=1) \
                                    .broadcast_to([128, I, B])
                                nc.vector.tensor_mul(sXtN[:],
                                                     sXT3[:, :, ntp, :], zb)
                                for cb in range(4):
                                    eb = sET[:, ntp, cb * 8:(cb + 1) * 8, :] \
                                        .rearrange("p (o c) b -> p o c b", o=1) \
                                        .broadcast_to([128, I, 8, B])
                                    xb = sXtN[:] \
                                        .rearrange("p i (o b) -> p i o b", o=1) \
                                        .broadcast_to([128, I, 8, B])
                                    nc.vector.tensor_mul(sRgN[cb][:], eb, xb)
                # (d) t=0: uniform couplings 1/C — plain W3 x xt3 matmuls
                if t == 0:
                    with tc.tile_pool(name="psS0", bufs=1, space="PSUM") as psS0:
                        for cb in range(4):
                            pacc0 = psS0.tile([128, B], f32, tag="s8")
                            step = 0
                            for i in range(I):
                                for nt in range(NT):
                                    lhs = sW3[:, cb, nt, i, :, :] \
                                        .rearrange("p a b -> p (a b)")
                                    nc.tensor.matmul(
                                        pacc0[:], lhs, sXT3[:, i, nt, :],
                                        start=(step == 0), stop=(step == 31))
                                    step += 1
                            nc.scalar.mul(sST[:, cb, :], pacc0[:], 1.0 / C)
                # all-reduce partial s across cores; t=0 uses the transposed
                # sST layout (+ PE transposes back), t>0 reduces [64, 512].
                if t == 0:
                    if not sim:
                        di = dp.tile([128, 4 * B], f32, tag="ar_in")
                        do = dp.tile([128, 4 * B], f32, tag="ar_out")
                        nc.sync.dma_start(di[:], sST[:].rearrange("p a b -> p (a b)"))
                        nc.gpsimd.collective_compute(
                            "AllReduce", mybir.AluOpType.add,
                            replica_groups=[list(range(NCORES))],
                            ins=[di[:].opt()], outs=[do[:].opt()])
                        nc.sync.dma_start(sSTr[:].rearrange("p a b -> p (a b)"), do[:])
                    with tc.tile_pool(name=f"psT{t}", bufs=2, space="PSUM") as psT:
                        for cb in range(4):
                            pT3 = psT.tile([B, 128], f32, tag="sT")
                            nc.tensor.transpose(pT3[:], sSTr[:, cb, :], sEyeF[:])
                            nc.scalar.copy(sS[:, cb * 128:(cb + 1) * 128], pT3[:])
                else:
                    if sim:
                        nc.vector.tensor_copy(sS[:], sSpre[:])
                    else:
                        di2 = dp.tile([B, C * D], f32, tag="ar_in2")
                        do2 = dp.tile([B, C * D], f32, tag="ar_out2")
                        nc.sync.dma_start(di2[:], sSpre[:])
                        nc.gpsimd.collective_compute(
                            "AllReduce", mybir.AluOpType.add,
                            replica_groups=[list(range(NCORES))],
                            ins=[di2[:].opt()], outs=[do2[:].opt()])
                        nc.sync.dma_start(sS[:], do2[:])
                if t > 0 and "dbg" in ablate:
                    nc.sync.dma_start(
                        dbgE_d[:], sET[:].rearrange("p a b c -> p (a b c)"))
                    nc.sync.dma_start(dbgS_d[:], sSpre[:])
                squash(sS, sOut)
                if t == n_rout - 1:
                    nc.sync.dma_start(out_d[:], sOut[:])
                elif t == 0:
                    for h in range(2):
                        sl = slice(h * C * D // 2, (h + 1) * C * D // 2)
                        nc.gpsimd.tensor_copy(sOsum[:, sl], sOut[:, sl])
                else:
                    for h in range(2):
                        sl = slice(h * C * D // 2, (h + 1) * C * D // 2)
                        nc.gpsimd.tensor_add(sOsum[:, sl], sOsum[:, sl],
                                             sOut[:, sl])
    nc.compile()
    return nc


def get_nc(sim=False, ablate=()):
    key = ("nc_sim" if sim else "nc") + "_".join(ablate)
    if key not in _CACHE:
        _CACHE[key] = _build_nc(sim=sim, ablate=ablate)
    return _CACHE[key]


def kernel(inputs, W):
    inputs = np.asarray(inputs, dtype=np.float32)
    W = np.asarray(W, dtype=np.float32)
    nc = get_nc()
    in_maps = host_prep_all(inputs, W)
    from concourse import bass_utils
    res = bass_utils.run_bass_kernel_spmd(
        nc, in_maps, core_ids=list(range(NCORES)))
    return res.results[0]["out"].reshape(B, C, D).astype(np.float32)
